# revision 1
# baseline (speedup 1.0000x reference)
"""DNC forward kernel for 8 Trainium2 NeuronCores.

Data-parallel over batch (B=16 -> 8 cores x 2 batch elements). The entire
32-step DNC recurrence runs on-device in a single Bass/Tile kernel per core
with all state (memory matrix, temporal link matrices, usage, read/write
weights, LSTM state) resident in SBUF. The host only prepares transposed
weight layouts, the precomputed input projection X[t] @ w_ih_x.T (+bias),
and gathers the outputs.

Device-side formulations:
  - allocation weighting: sort-free. prod_{j before i} u_j =
    exp(sum_j cmp[j,i] * ln(u_j)) where cmp[j,i] = (u'_i > u'_j) on
    index-perturbed u' = u * (1 + idx*4e-7); the multiplicative perturbation
    reproduces the reference's stable (index-order) tie-breaking for
    exactly-equal usage values while changing real comparisons only for
    pairs within ~2e-4 relative (negligible product error).
  - the NxN compare matrix is built by a K=2 PE matmul (outer difference),
    and the masked log-sum is a PE matmul against the 0/1 compare matrix.
  - link matrix kept in both orientations (link, linkT), updated in place
    elementwise; diagonal zeroing via gpsimd.affine_select.

ISA partition rule: every operand AP must start at partition 0/32/64/96,
so per-batch rows live at partitions 0 (b=0) and 32 (b=1), per-(r,b)
scalar vectors at quadrant bases, and per-step outputs go along the free
dimension.
"""

import os
import numpy as np

B, S, IN, H = 16, 32, 256, 512
N, W, R = 512, 64, 4
OUT = 128
EPS = 1e-6
NCORES = 8
BL = B // NCORES          # 2 batch elements per core
G = 4 * H                 # 2048 LSTM gates
KN = N // 128             # 4 n-blocks
IFACE = 471

_NC_CACHE = {}


def _build_nc(n_steps=S, dbg=False):
    import concourse.bass as bass
    import concourse.mybir as mybir
    import concourse.tile as tile

    f32 = mybir.dt.float32
    AF = mybir.ActivationFunctionType
    ALU = mybir.AluOpType
    AX = mybir.AxisListType

    import concourse.bacc as bacc
    nc = bacc.Bacc("TRN2", target_bir_lowering=False, debug=False)

    def dram(name, shape, kind="ExternalInput"):
        return nc.dram_tensor(name, shape, f32, kind=kind)

    d_xp = dram("xp", [S, BL, G])           # x @ w_ih[:, :IN].T + b_ih + b_hh
    d_whh = dram("whh", [128, 4, G])        # w_hh.T in k-blocks
    d_wir = dram("wir", [128, 2, G])        # w_ih[:, IN:].T packed (rm*64+w, rh)
    d_wif = dram("wif", [128, 4, IFACE])    # W_iface in k-blocks
    d_bif = dram("bif", [BL, IFACE])
    d_woh = dram("woh", [128, 4, OUT])      # W_out[:H] in k-blocks
    d_wor = dram("wor", [128, 2, OUT])      # W_out[H:] packed (rm*64+w, rh)
    d_bo = dram("bo", [BL, OUT])
    d_pert = dram("pert", [1, N])           # 1 + n*4e-7 tie-break perturbation
    d_id = dram("ident", [128, 128])
    d_outs = dram("outs", [S, BL, OUT], kind="ExternalOutput")
    if dbg:
        d_du = dram("dbg_u", [33, N], kind="ExternalOutput")
        d_dwp = dram("dbg_wp", [33, N], kind="ExternalOutput")
        d_dpr = dram("dbg_pr", [33, N], kind="ExternalOutput")
        d_drw = dram("dbg_rw", [8, N], kind="ExternalOutput")
        d_dmem = dram("dbg_mem", [64, BL, N], kind="ExternalOutput")
        d_dcc = dram("dbg_cc", [BL, H], kind="ExternalOutput")
        d_dlink = dram("dbg_link", [128, BL, KN, N], kind="ExternalOutput")
        d_dlinkT = dram("dbg_linkT", [128, BL, KN, N], kind="ExternalOutput")

    with tile.TileContext(nc) as tc:
        with (
            tc.tile_pool(name="cst", bufs=1) as cst,
            tc.tile_pool(name="st", bufs=1) as st,
            tc.tile_pool(name="tmp", bufs=1) as tmp,
            tc.tile_pool(name="tm2", bufs=2) as tm2,
            tc.tile_pool(name="tm3", bufs=2) as tm3,
            tc.tile_pool(name="tb1", bufs=1) as tb1,
            tc.tile_pool(name="psK", bufs=2, space="PSUM") as psK,
            tc.tile_pool(name="psR", bufs=2, space="PSUM") as psR,
            tc.tile_pool(name="psF", bufs=2, space="PSUM") as psF,
        ):
            # ---------------- load weights / constants ----------------
            def load(d, shape, tag):
                t = cst.tile(shape, f32, tag=tag)
                nc.sync.dma_start(out=t[...], in_=d[...])
                return t

            whh = load(d_whh, [128, 4, G], "whh")
            wir = load(d_wir, [128, 2, G], "wir")
            wif = load(d_wif, [128, 4, IFACE], "wif")
            bif = load(d_bif, [BL, IFACE], "bif")
            woh = load(d_woh, [128, 4, OUT], "woh")
            wor = load(d_wor, [128, 2, OUT], "wor")
            bo = load(d_bo, [BL, OUT], "bo")
            pert = cst.tile([33, N], f32, tag="pert")
            nc.vector.memset(pert[...], 0.0)
            nc.sync.dma_start(out=pert[0:1, :], in_=d_pert[...])
            nc.sync.dma_start(out=pert[32:33, :], in_=d_pert[...])
            ident = load(d_id, [128, 128], "ident")

            onesq = cst.tile([128, 128], f32)
            nc.vector.memset(onesq[...], 1.0)
            ones64 = cst.tile([64, 1], f32)
            nc.vector.memset(ones64[...], 1.0)

            # ---------------- persistent state ----------------
            # per-batch rows: b=0 at partition 0, b=1 at partition 32.
            hT = st.tile([128, KN, BL], f32)       # h transposed (k-blocks)
            cc = st.tile([BL, H], f32)             # LSTM cell
            memT = st.tile([64, BL, N], f32)       # memory transposed (w, b, n)
            link = st.tile([128, BL, KN, N], f32)  # link[i-part, j-free]
            linkT = st.tile([128, BL, KN, N], f32)  # link[j-part, i-free]
            u = st.tile([33, N], f32)              # usage rows 0/32
            WPa = st.tile([33, N], f32)            # write_w rows 0/32
            PRC = st.tile([33, N], f32)            # precedence rows 0/32
            rww0 = st.tile([4, N], f32)            # read_w b=0 rows r
            rww1 = st.tile([4, N], f32)            # read_w b=1 rows r
            rwws = (rww0, rww1)
            rwT = st.tile([128, KN, 4 * BL], f32)  # read_w transposed
            rwd = st.tile([128, 2, BL], f32)       # read_words^T packed (rm*64+w, rh, b)
            UBP = st.tile([33, N], f32)            # ubar*pert rows 0/32
            NUP = st.tile([33, N], f32)            # -ubar*pert rows 0/32
            NWr = st.tile([33, N], f32)            # -w rows 0/32
            OMW = st.tile([33, N], f32)            # 1-w rows 0/32
            psiS = st.tile([33, N], f32)           # per-step psi (persistent for init marking)
            omuS = st.tile([33, N], f32)
            ubarS = st.tile([33, N], f32)
            omubS = st.tile([33, N], f32)
            alcS = st.tile([33, N], f32)
            outs_sb = st.tile([BL, S * OUT], f32)

            for t_ in (hT, cc, memT, link, linkT, u, WPa, PRC, rww0, rww1, rwT, rwd,
                       UBP, NUP, NWr, OMW, psiS, omuS, ubarS, omubS, alcS):
                nc.vector.memset(t_[...], 0.0)
            onesN = cst.tile([33, N], f32)
            nc.vector.memset(onesN[...], 1.0)

            wrow = lambda b: WPa[32 * b:32 * b + 1, :]
            r32 = lambda t_, b: t_[32 * b:32 * b + 1, :]

            MM = nc.tensor.matmul

            def TP(out, in_):
                p = in_.shape[0]
                q = in_.base_partition()
                nc.tensor.transpose(out, in_, ident[q:q + p, q:q + p])

            # ---------------- one recurrence step ----------------
            def emit_step(t):
                # stream this step's input projection from DRAM
                xpt = tm3.tile([BL, G], f32, tag="xpt")
                nc.sync.dma_start(out=xpt[...], in_=d_xp[t, :, :])

                # ===== A: LSTM gates =====
                gsb = tb1.tile([BL, G], f32, tag="gsb")
                for c in range(4):
                    gp = psR.tile([BL, 512], f32, tag="row")
                    for k in range(4):
                        MM(gp[...], hT[:, k, :], whh[:, k, c * 512:(c + 1) * 512],
                           start=(k == 0), stop=False)
                    for rh in range(2):
                        MM(gp[...], rwd[:, rh, :], wir[:, rh, c * 512:(c + 1) * 512],
                           start=False, stop=(rh == 1))
                    nc.vector.tensor_tensor(
                        gsb[:, c * 512:(c + 1) * 512], gp[...],
                        xpt[:, c * 512:(c + 1) * 512], op=ALU.add)

                sif = tmp.tile([BL, 2 * H], f32, tag="sif")
                nc.scalar.activation(sif[...], gsb[:, 0:1024], AF.Sigmoid)
                tg = tmp.tile([BL, H], f32, tag="tg")
                nc.scalar.activation(tg[...], gsb[:, 1024:1536], AF.Tanh)
                so = tmp.tile([BL, H], f32, tag="so")
                nc.scalar.activation(so[...], gsb[:, 1536:2048], AF.Sigmoid)
                nc.vector.tensor_tensor(cc[...], cc[...], sif[:, 512:1024], op=ALU.mult)
                t1 = tmp.tile([BL, H], f32, tag="t1")
                nc.vector.tensor_tensor(t1[...], sif[:, 0:512], tg[...], op=ALU.mult)
                nc.vector.tensor_tensor(cc[...], cc[...], t1[...], op=ALU.add)
                hrow = tmp.tile([BL, H], f32, tag="hrow")
                nc.scalar.activation(hrow[...], cc[...], AF.Tanh)
                nc.vector.tensor_tensor(hrow[...], hrow[...], so[...], op=ALU.mult)
                # h transpose (gates above already consumed old hT)
                hp = psK.tile([128, KN * BL], f32, tag="kk")
                for k in range(4):
                    TP(hp[:, k * BL:(k + 1) * BL], hrow[:, k * 128:(k + 1) * 128])
                nc.vector.tensor_copy(hT[...], hp[...])

                # ===== interface =====
                ifp = psR.tile([BL, IFACE], f32, tag="row")
                for k in range(4):
                    MM(ifp[...], hT[:, k, :], wif[:, k, :], start=(k == 0), stop=(k == 3))
                ifc = tb1.tile([BL, IFACE], f32, tag="gsb")
                nc.vector.tensor_tensor(ifc[...], ifp[...], bif[...], op=ALU.add)
                # read-modes softmax over the 3 modes (cols 459:471)
                mv = ifc[:, 459:471].rearrange("p (r j) -> p r j", j=3)
                mmax = tmp.tile([BL, 4], f32, tag="mmax")
                nc.vector.tensor_reduce(mmax[...], mv, axis=AX.X, op=ALU.max, negate=True)
                nc.vector.tensor_tensor(mv, mv, mmax[:, :, None].broadcast_to([BL, 4, 3]),
                                        op=ALU.add)
                nc.scalar.activation(mv, mv, AF.Exp)
                msum = tmp.tile([BL, 4], f32, tag="msum")
                nc.vector.tensor_reduce(msum[...], mv, axis=AX.X, op=ALU.add)
                nc.vector.reciprocal(msum[...], msum[...])
                nc.vector.tensor_tensor(mv, mv, msum[:, :, None].broadcast_to([BL, 4, 3]),
                                        op=ALU.mult)

                # ===== transpose interface pieces =====
                # walrus requires matmul PSUM outputs at partition 0, so each
                # transposed piece gets its own column pair of tpS.
                # col pairs: 0 wkey[0:64], 1 erase[0:64], 2 wvc[0:64],
                #            3 rstr[0:4], 4 wstr[0:1], 5 fg[0:4], 6 m0[0:4],
                #            7 m1[0:4], 8 m2[0:4], 9 ag[0:1], 10 wg[0:1]
                tpS = psK.tile([64, 22], f32, tag="kk")
                TP(tpS[0:64, 0:2], ifc[:, 260:324])
                TP(tpS[0:64, 2:4], ifc[:, 325:389])
                TP(tpS[0:64, 4:6], ifc[:, 389:453])
                TP(tpS[0:4, 6:8], ifc[:, 256:260])
                TP(tpS[0:1, 8:10], ifc[:, 324:325])
                TP(tpS[0:4, 10:12], ifc[:, 453:457])
                TP(tpS[0:4, 12:14], mv[:, :, 0])
                TP(tpS[0:4, 14:16], mv[:, :, 1])
                TP(tpS[0:4, 16:18], mv[:, :, 2])
                TP(tpS[0:1, 18:20], ifc[:, 457:458])
                TP(tpS[0:1, 20:22], ifc[:, 458:459])

                ersT = tmp.tile([64, BL], f32, tag="ersT")
                nc.scalar.activation(ersT[...], tpS[0:64, 2:4], AF.Sigmoid)
                wvcT = tmp.tile([64, BL], f32, tag="wvcT")
                nc.scalar.activation(wvcT[...], tpS[0:64, 4:6], AF.Sigmoid)
                # SC: rows 0-3 fg (sigmoid), 32-35 m0, 64-67 m1, 96-99 m2
                SC = tmp.tile([128, BL], f32, tag="SC")
                nc.scalar.activation(SC[0:4, :], tpS[0:4, 10:12], AF.Sigmoid)
                nc.vector.tensor_copy(SC[32:36, :], tpS[0:4, 12:14])
                nc.vector.tensor_copy(SC[64:68, :], tpS[0:4, 14:16])
                nc.vector.tensor_copy(SC[96:100, :], tpS[0:4, 16:18])
                # SC2: rows 0-3 oneplus(read_str), 32 oneplus(write_str),
                #      64 alloc_gate (sigmoid), 96 write_gate (sigmoid)
                SC2 = tmp.tile([128, BL], f32, tag="SC2")

                def oneplus(dst, src, tmptag):
                    # dst = 1 + softplus(src) = 1 + max(src,0) + ln(1+e^-|src|)
                    # computed fully at partition base 0, then copied to dst
                    # (two-SBUF-input ops must share the input base partition).
                    a = tmp.tile([4, 2 * BL], f32, tag=tmptag)
                    p = src.shape[0]
                    nc.scalar.activation(a[0:p, 0:BL], src, AF.Abs)
                    nc.scalar.activation(a[0:p, 0:BL], a[0:p, 0:BL], AF.Exp, scale=-1.0)
                    nc.vector.tensor_scalar(a[0:p, 0:BL], a[0:p, 0:BL], 1.0, None,
                                            op0=ALU.add)
                    nc.scalar.activation(a[0:p, 0:BL], a[0:p, 0:BL], AF.Ln)
                    nc.vector.tensor_scalar(a[0:p, BL:2 * BL], src, 0.0, 1.0,
                                            op0=ALU.max, op1=ALU.add)
                    nc.vector.tensor_tensor(a[0:p, BL:2 * BL], a[0:p, BL:2 * BL],
                                            a[0:p, 0:BL], op=ALU.add)
                    nc.scalar.activation(dst, a[0:p, BL:2 * BL], AF.Copy)

                oneplus(SC2[0:4, :], tpS[0:4, 6:8], "opa")
                oneplus(SC2[32:33, :], tpS[0:1, 8:10], "opb")
                nc.scalar.activation(SC2[64:65, :], tpS[0:1, 18:20], AF.Sigmoid)
                nc.scalar.activation(SC2[96:97, :], tpS[0:1, 20:22], AF.Sigmoid)
                # 1 - alloc_gate
                omag = tmp.tile([1, BL], f32, tag="omag")
                nc.scalar.activation(omag[...], SC2[64:65, :], AF.Copy,
                                     scale=-1.0, bias=1.0)

                # keysT [64, b, 5]: read keys 0-3, write key 4
                keysT = tmp.tile([64, BL, 5], f32, tag="keysT")
                kp = psK.tile([64, 4 * BL], f32, tag="kk")
                for r in range(4):
                    TP(kp[:, r * BL:(r + 1) * BL], ifc[:, r * 64:(r + 1) * 64])
                nc.vector.tensor_copy(
                    keysT[:, :, 0:4], kp[:, :].rearrange("p (r b) -> p b r", b=BL))
                nc.vector.tensor_copy(keysT[:, :, 4:5], tpS[0:64, 0:2, None])

                # key norms -> nkcol rows: 0-3 read b0, 32-35 read b1,
                #                          64 write b0, 96 write b1
                kq = tmp.tile([64, BL * 5], f32, tag="kq")
                nc.scalar.activation(kq[...], keysT[:, :, :].rearrange("p a b -> p (a b)"),
                                     AF.Square)
                nkp = psK.tile([4, 16], f32, tag="kk")
                MM(nkp[0:1, 0:BL * 5], ones64[...], kq[...], start=True, stop=True)
                nks = tmp.tile([1, BL * 5], f32, tag="nks")
                nc.scalar.activation(nks[...], nkp[0:1, 0:BL * 5], AF.Sqrt)
                TP(nkp[0:4, 10:11], nks[:, 0:4])      # read b0
                TP(nkp[0:4, 11:12], nks[:, 5:9])      # read b1
                TP(nkp[0:1, 12:13], nks[:, 4:5])      # write b0
                TP(nkp[0:1, 13:14], nks[:, 9:10])     # write b1
                nkcol = tmp.tile([97, 1], f32, tag="nkcol")
                nc.vector.tensor_copy(nkcol[0:4, :], nkp[0:4, 10:11])
                nc.vector.tensor_copy(nkcol[32:36, :], nkp[0:4, 11:12])
                nc.vector.tensor_copy(nkcol[64:65, :], nkp[0:1, 12:13])
                nc.vector.tensor_copy(nkcol[96:97, :], nkp[0:1, 13:14])

                # ===== write content weights (old memory) =====
                sqm = tb1.tile([64, BL * N], f32, tag="gsb")
                nc.scalar.activation(sqm[...], memT[:, :, :].rearrange("p a b -> p (a b)"),
                                     AF.Square)
                nmp = psR.tile([1, BL, N], f32, tag="row")
                for b in range(BL):
                    MM(nmp[0:1, b, :], ones64[...], sqm[:, b * N:(b + 1) * N],
                       start=True, stop=True)
                nmo = tmp.tile([33, N], f32, tag="nmo")
                for b in range(BL):
                    nc.scalar.activation(nmo[32 * b:32 * b + 1, :],
                                         nmp[0:1, b, :], AF.Sqrt)
                wdp = psR.tile([1, BL, N], f32, tag="row")
                for b in range(BL):
                    MM(wdp[0:1, b, :], keysT[:, b, 4:5], memT[:, b, :],
                       start=True, stop=True)
                cosw = tmp.tile([33, N], f32, tag="cosw")
                for b in range(BL):
                    den = tmp.tile([1, N], f32, tag="den")
                    nc.vector.tensor_scalar(den[...], nmo[32 * b:32 * b + 1, :],
                                            nkcol[64 + 32 * b:65 + 32 * b, :], EPS,
                                            op0=ALU.mult, op1=ALU.add)
                    nc.vector.reciprocal(den[...], den[...])
                    nc.vector.tensor_tensor(den[...], wdp[0:1, b, :], den[...],
                                            op=ALU.mult)
                    nc.vector.tensor_scalar(cosw[32 * b:32 * b + 1, :], den[...],
                                            SC2[32:33, b:b + 1], None, op0=ALU.mult)
                wmx = tmp.tile([33, 1], f32, tag="wmx")
                wsm = tmp.tile([33, 1], f32, tag="wsm")
                for b in range(BL):
                    nc.vector.tensor_reduce(r32(wmx, b)[:, 0:1], r32(cosw, b),
                                            axis=AX.X, op=ALU.max, negate=True)
                    nc.scalar.activation(r32(cosw, b), r32(cosw, b), AF.Exp,
                                         bias=wmx[32 * b:32 * b + 1, 0:1])
                    nc.vector.tensor_reduce(r32(wsm, b)[:, 0:1], r32(cosw, b),
                                            axis=AX.X, op=ALU.add)
                    nc.vector.reciprocal(r32(wsm, b)[:, 0:1], r32(wsm, b)[:, 0:1])
                cw = tmp.tile([33, N], f32, tag="cw")
                for b in range(BL):
                    nc.vector.tensor_scalar(cw[32 * b:32 * b + 1, :],
                                            cosw[32 * b:32 * b + 1, :],
                                            wsm[32 * b:32 * b + 1, :], None,
                                            op0=ALU.mult)

                # ===== psi and usage =====
                nfg = tmp.tile([4, BL], f32, tag="nfg")
                nc.scalar.activation(nfg[...], SC[0:4, :], AF.Copy, scale=-1.0)
                pst = tmp.tile([36, N], f32, tag="pst")
                for b in range(BL):
                    nc.vector.tensor_scalar(pst[32 * b:32 * b + 4, :],
                                            rwws[b][0:4, :],
                                            nfg[:, b:b + 1], 1.0, op0=ALU.mult, op1=ALU.add)
                    nc.scalar.activation(pst[32 * b:32 * b + 4, :],
                                         pst[32 * b:32 * b + 4, :], AF.Ln)
                psp = psR.tile([1, BL, N], f32, tag="row")
                for b in range(BL):
                    MM(psp[0:1, b, :], ones64[32 * b:32 * b + 4, :],
                       pst[32 * b:32 * b + 4, :], start=True, stop=True)
                psi = psiS
                omu = omuS
                for b in range(BL):
                    nc.scalar.activation(r32(psi, b), psp[0:1, b, :], AF.Exp)
                # u = (u + w_prev*(1-u)) * psi, full-tile (rows 1-31 are dead)
                nc.scalar.activation(omu[...], u[...], AF.Copy, scale=-1.0, bias=1.0)
                nc.vector.tensor_tensor(omu[...], omu[...], WPa[...], op=ALU.mult)
                nc.vector.tensor_tensor(u[...], u[...], omu[...], op=ALU.add)
                nc.vector.tensor_tensor(u[...], u[...], psi[...], op=ALU.mult)

                # ===== allocation =====
                ubar = ubarS
                omub = omubS
                nc.scalar.activation(ubar[...], u[...], AF.Copy,
                                     scale=float(1.0 - EPS), bias=float(EPS))
                nc.vector.tensor_tensor(UBP[...], ubar[...], pert[...], op=ALU.mult)
                nc.scalar.activation(NUP[...], UBP[...], AF.Copy, scale=-1.0)
                nc.scalar.activation(omub[...], ubar[...], AF.Copy,
                                     scale=-1.0, bias=1.0)
                up = psK.tile([128, KN * BL], f32, tag="kk")
                for b in range(BL):
                    for k in range(4):
                        TP(up[:, k * BL + b:k * BL + b + 1],
                           ubar[32 * b:32 * b + 1, k * 128:(k + 1) * 128])
                logu = tmp.tile([128, KN, BL], f32, tag="logu")
                nc.scalar.activation(logu[...], up[...], AF.Ln)
                lpp = psR.tile([1, BL, N], f32, tag="row")
                for b in range(BL):
                    for k in range(4):
                        dp = psK.tile([128, N], f32, tag="kk")
                        # D[j, i] = ubar'_i - ubar'_j via two K=1 matmuls
                        MM(dp[...], NUP[32 * b:32 * b + 1, k * 128:(k + 1) * 128],
                           r32(onesN, b), start=True, stop=False)
                        MM(dp[...], onesq[32 * b:32 * b + 1, :], r32(UBP, b),
                           start=False, stop=True)
                        cmp_ = tm2.tile([128, N], f32, tag="cmp")
                        nc.vector.tensor_scalar(cmp_[...], dp[...], 0.0, None,
                                                op0=ALU.is_gt)
                        MM(lpp[0:1, b, :], logu[:, k, b:b + 1], cmp_[...],
                           start=(k == 0), stop=(k == 3))
                alc = alcS
                for b in range(BL):
                    nc.scalar.activation(r32(alc, b), lpp[0:1, b, :], AF.Exp)
                nc.vector.tensor_tensor(alc[...], alc[...], omub[...], op=ALU.mult)

                # ===== write weights =====
                wrB = tmp.tile([33, N], f32, tag="den")
                agB = tmp.tile([33, N], f32, tag="pst")
                for b in range(BL):
                    nc.vector.tensor_scalar(r32(wrB, b), cw[32 * b:32 * b + 1, :],
                                            omag[:, b:b + 1], None, op0=ALU.mult)
                    nc.vector.tensor_scalar(r32(agB, b), alc[32 * b:32 * b + 1, :],
                                            SC2[64:65, b:b + 1], None, op0=ALU.mult)
                    nc.vector.tensor_tensor(r32(wrB, b), r32(wrB, b), r32(agB, b),
                                            op=ALU.add)
                    nc.vector.tensor_scalar(wrow(b), r32(wrB, b), SC2[96:97, b:b + 1],
                                            None, op0=ALU.mult)

                # transposed write/prec columns: wpc cols (w0, w1, p0, p1)
                wpp = psK.tile([128, KN * 4], f32, tag="kk")
                for k in range(4):
                    for b in range(BL):
                        TP(wpp[:, k * 4 + b:k * 4 + b + 1],
                           WPa[32 * b:32 * b + 1, k * 128:(k + 1) * 128])
                        TP(wpp[:, k * 4 + 2 + b:k * 4 + 3 + b],
                           PRC[32 * b:32 * b + 1, k * 128:(k + 1) * 128])
                wpc = tmp.tile([128, KN, 4], f32, tag="wpc")
                nc.vector.tensor_copy(wpc[...], wpp[...])
                # broadcasts of w and prec along partitions -> wpb[:, 0:2]=w, 2:4=prec
                wpb = tb1.tile([128, 4, N], f32, tag="gsb")
                for i in range(4):
                    src = wrow(i) if i < 2 else PRC[32 * (i - 2):32 * (i - 2) + 1, :]
                    bp = psK.tile([128, N], f32, tag="kk")
                    MM(bp[...], onesq[32 * (i % 2):32 * (i % 2) + 1, :], src,
                       start=True, stop=True)
                    nc.vector.tensor_copy(wpb[:, i, :], bp[...])
                # A-matrix operands and prec update, full-tile
                swr = tmp.tile([33, 1], f32, tag="swr")
                nc.scalar.activation(NWr[...], WPa[...], AF.Copy, scale=-1.0)
                nc.scalar.activation(OMW[...], WPa[...], AF.Copy,
                                     scale=-1.0, bias=1.0)
                nc.vector.tensor_reduce(swr[...], WPa[...], axis=AX.X, op=ALU.add,
                                        negate=True)
                nc.vector.tensor_scalar(swr[...], swr[...], 1.0, None, op0=ALU.add)
                nc.vector.scalar_tensor_tensor(PRC[...], PRC[...], swr[...], WPa[...],
                                               op0=ALU.mult, op1=ALU.add)

                # ===== memory update (memT) =====
                for b in range(BL):
                    ew = tmp.tile([64, N], f32, tag="ew")
                    nc.gpsimd.tensor_scalar(ew[...], wpb[0:64, b, :], ersT[:, b:b + 1],
                                            None, op0=ALU.mult)
                    em = tmp.tile([64, N], f32, tag="em")
                    nc.gpsimd.tensor_tensor(em[...], memT[:, b, :], ew[...], op=ALU.mult)
                    nc.gpsimd.tensor_tensor(memT[:, b, :], memT[:, b, :], em[...],
                                            op=ALU.subtract)
                    ww = tmp.tile([64, N], f32, tag="ew")
                    nc.gpsimd.tensor_scalar(ww[...], wpb[0:64, b, :], wvcT[:, b:b + 1],
                                            None, op0=ALU.mult)
                    nc.gpsimd.tensor_tensor(memT[:, b, :], memT[:, b, :], ww[...],
                                            op=ALU.add)

                # mem in [n, w] orientation (for read words)
                mnw = tm2.tile([128, BL, KN, 64], f32, tag="cmp")
                for b in range(BL):
                    mp = psK.tile([128, KN * 64], f32, tag="kk")
                    for k in range(4):
                        TP(mp[:, k * 64:(k + 1) * 64], memT[:, b, k * 128:(k + 1) * 128])
                    nc.vector.tensor_copy(mnw[:, b, :, :], mp[...])

                # ===== link matrices =====
                for b in range(BL):
                    for k in range(4):
                        ap_ = psK.tile([128, N], f32, tag="kk")
                        # A[n, m] = 1 - w_n - w_m via two K=1 matmuls
                        MM(ap_[...], NWr[32 * b:32 * b + 1, k * 128:(k + 1) * 128],
                           r32(onesN, b), start=True, stop=False)
                        MM(ap_[...], onesq[32 * b:32 * b + 1, :], r32(OMW, b),
                           start=False, stop=True)
                        L = link[:, b, k, :]
                        nc.vector.tensor_tensor(L, L, ap_[...], op=ALU.mult)
                        nc.vector.scalar_tensor_tensor(L, wpb[:, 2 + b, :],
                                                       wpc[:, k, b:b + 1], L,
                                                       op0=ALU.mult, op1=ALU.add)
                        nc.gpsimd.affine_select(L, L, [[1, N]], ALU.not_equal, 0.0,
                                                base=-(128 * k), channel_multiplier=-1)
                        LT = linkT[:, b, k, :]
                        nc.vector.tensor_tensor(LT, LT, ap_[...], op=ALU.mult)
                        nc.vector.scalar_tensor_tensor(LT, wpb[:, b, :],
                                                       wpc[:, k, 2 + b:3 + b], LT,
                                                       op0=ALU.mult, op1=ALU.add)
                        nc.gpsimd.affine_select(LT, LT, [[1, N]], ALU.not_equal, 0.0,
                                                base=-(128 * k), channel_multiplier=-1)

                # ===== forward/backward read weights =====
                fbs = []
                for b in range(BL):
                    fb = psF.tile([4, N], f32, tag="fbb")
                    for k in range(4):
                        MM(fb[...], rwT[:, k, 4 * b:4 * b + 4], linkT[:, b, k, :],
                           start=(k == 0), stop=(k == 3))
                    bb = psF.tile([4, N], f32, tag="fbb")
                    for k in range(4):
                        MM(bb[...], rwT[:, k, 4 * b:4 * b + 4], link[:, b, k, :],
                           start=(k == 0), stop=(k == 3))
                    fbs.append((fb, bb))

                # ===== read content weights (new memory) =====
                sqn = tb1.tile([64, BL * N], f32, tag="gsb")
                nc.scalar.activation(sqn[...], memT[:, :, :].rearrange("p a b -> p (a b)"),
                                     AF.Square)
                nmp2 = psR.tile([1, BL, N], f32, tag="row")
                for b in range(BL):
                    MM(nmp2[0:1, b, :], ones64[...], sqn[:, b * N:(b + 1) * N],
                       start=True, stop=True)
                nmn = tmp.tile([33, N], f32, tag="nmo")
                for b in range(BL):
                    nc.scalar.activation(nmn[32 * b:32 * b + 1, :],
                                         nmp2[0:1, b, :], AF.Sqrt)
                cr8 = tmp.tile([36, N], f32, tag="pst")
                for b in range(BL):
                    rdp = psK.tile([4, N], f32, tag="kk")
                    MM(rdp[...], keysT[:, b, 0:4], memT[:, b, :], start=True, stop=True)
                    n4p = psK.tile([4, N], f32, tag="kk")
                    MM(n4p[...], onesq[32 * b:32 * b + 1, 0:4],
                       nmn[32 * b:32 * b + 1, :], start=True, stop=True)
                    den4 = tmp.tile([4, N], f32, tag="den")
                    nc.vector.tensor_scalar(den4[...], n4p[...],
                                            nkcol[32 * b:32 * b + 4, :], EPS,
                                            op0=ALU.mult, op1=ALU.add)
                    nc.vector.reciprocal(den4[...], den4[...])
                    nc.vector.tensor_tensor(den4[...], rdp[...], den4[...], op=ALU.mult)
                    nc.vector.tensor_scalar(cr8[32 * b:32 * b + 4, :], den4[...],
                                            SC2[0:4, b:b + 1], None, op0=ALU.mult)
                    rmx = tmp.tile([4, 1], f32, tag="rmx")
                    nc.vector.tensor_reduce(rmx[...], cr8[32 * b:32 * b + 4, :],
                                            axis=AX.X, op=ALU.max, negate=True)
                    nc.scalar.activation(cr8[32 * b:32 * b + 4, :],
                                         cr8[32 * b:32 * b + 4, :], AF.Exp, bias=rmx[...])
                    rsm = tmp.tile([4, 1], f32, tag="rsm")
                    nc.vector.tensor_reduce(rsm[...], cr8[32 * b:32 * b + 4, :],
                                            axis=AX.X, op=ALU.add)
                    nc.vector.reciprocal(rsm[...], rsm[...])
                    nc.vector.tensor_scalar(cr8[32 * b:32 * b + 4, :],
                                            cr8[32 * b:32 * b + 4, :], rsm[...], None,
                                            op0=ALU.mult)

                # ===== mix read weights: bwd*m0 + cr*m1 + fwd*m2 =====
                for b in range(BL):
                    fb, bb = fbs[b]
                    mixt = tm2.tile([4, N], f32, tag="mixt")
                    mixu = tm2.tile([4, N], f32, tag="cmp")
                    nc.vector.tensor_scalar(mixt[...], cr8[32 * b:32 * b + 4, :],
                                            SC[64:68, b:b + 1], None, op0=ALU.mult)
                    nc.vector.tensor_scalar(mixu[...], bb[...],
                                            SC[32:36, b:b + 1], None, op0=ALU.mult)
                    nc.vector.tensor_tensor(mixt[...], mixt[...], mixu[...], op=ALU.add)
                    nc.vector.tensor_scalar(mixu[...], fb[...],
                                            SC[96:100, b:b + 1], None, op0=ALU.mult)
                    nc.vector.tensor_tensor(rwws[b][0:4, :], mixt[...],
                                            mixu[...], op=ALU.add)

                # read_w transpose for next step + read words
                rwpp = psK.tile([128, KN * 4 * BL], f32, tag="kk")
                for k in range(4):
                    for b in range(BL):
                        TP(rwpp[:, k * 8 + 4 * b:k * 8 + 4 * b + 4],
                           rwws[b][0:4, k * 128:(k + 1) * 128])
                nc.vector.tensor_copy(rwT[...], rwpp[...])

                for b in range(BL):
                    rwp = psK.tile([64, 4], f32, tag="kk")
                    for k in range(4):
                        MM(rwp[...], mnw[:, b, k, :], rwT[:, k, 4 * b:4 * b + 4],
                           start=(k == 0), stop=(k == 3))
                    nc.vector.tensor_copy(rwd[0:64, :, b:b + 1], rwp[:, 0:4:2, None])
                    nc.vector.tensor_copy(rwd[64:128, :, b:b + 1], rwp[:, 1:4:2, None])

                # ===== output =====
                outp = psK.tile([BL, OUT], f32, tag="kk")
                for k in range(4):
                    MM(outp[...], hT[:, k, :], woh[:, k, :], start=(k == 0), stop=False)
                for rh in range(2):
                    MM(outp[...], rwd[:, rh, :], wor[:, rh, :], start=False,
                       stop=(rh == 1))
                outv = tm2.tile([BL, OUT], f32, tag="outv")
                nc.vector.tensor_tensor(outv[...], outp[...], bo[...], op=ALU.add)
                nc.sync.dma_start(out=d_outs[t, :, :], in_=outv[...])

            for t in range(n_steps):
                emit_step(t)

            if dbg:
                nc.sync.dma_start(out=d_du[...], in_=u[...])
                nc.sync.dma_start(out=d_dwp[...], in_=WPa[...])
                nc.sync.dma_start(out=d_dpr[...], in_=PRC[...])
                nc.sync.dma_start(out=d_drw[0:4, :], in_=rww0[...])
                nc.sync.dma_start(out=d_drw[4:8, :], in_=rww1[...])
                nc.sync.dma_start(out=d_dmem[...], in_=memT[...])
                nc.sync.dma_start(out=d_dcc[...], in_=cc[...])
                nc.sync.dma_start(out=d_dlink[...], in_=link[...])
                nc.sync.dma_start(out=d_dlinkT[...], in_=linkT[...])

    nc.compile()
    return nc


def _host_prep(inputs, w_ih, w_hh, b_ih, b_hh, W_iface, b_iface, W_out, b_out):
    """Build the per-core input maps (transposed weight layouts)."""
    f32 = np.float32
    x = np.asarray(inputs, f32)
    w_ih = np.asarray(w_ih, f32)
    w_hh = np.asarray(w_hh, f32)
    bias = (np.asarray(b_ih, f32) + np.asarray(b_hh, f32))
    W_iface = np.asarray(W_iface, f32)
    b_iface = np.asarray(b_iface, f32)
    W_out = np.asarray(W_out, f32)
    b_out = np.asarray(b_out, f32)

    whh_h = np.ascontiguousarray(w_hh.T.reshape(4, 128, G).transpose(1, 0, 2))
    wir_h = np.ascontiguousarray(
        w_ih[:, IN:].T.reshape(2, 2, 64, G).transpose(1, 2, 0, 3).reshape(128, 2, G))
    wif_h = np.ascontiguousarray(W_iface.reshape(4, 128, IFACE).transpose(1, 0, 2))
    woh_h = np.ascontiguousarray(W_out[:H].reshape(4, 128, OUT).transpose(1, 0, 2))
    wor_h = np.ascontiguousarray(
        W_out[H:].reshape(2, 2, 64, OUT).transpose(1, 2, 0, 3).reshape(128, 2, OUT))
    bif_h = np.ascontiguousarray(np.tile(b_iface[None, :], (BL, 1)))
    bo_h = np.ascontiguousarray(np.tile(b_out[None, :], (BL, 1)))
    pert_h = np.ascontiguousarray(
        (1.0 + np.arange(N, dtype=f32) * np.float32(4e-7))[None, :].astype(f32))
    ident_h = np.eye(128, dtype=f32)

    wx = w_ih[:, :IN]
    in_maps = []
    for c in range(NCORES):
        xc = np.ascontiguousarray(x[:, c * BL:(c + 1) * BL, :]).reshape(S * BL, IN)
        xp_h = np.ascontiguousarray(
            (xc @ wx.T + bias[None, :]).reshape(S, BL, G)).astype(f32)
        in_maps.append({
            "xp": xp_h, "whh": whh_h, "wir": wir_h, "wif": wif_h, "bif": bif_h,
            "woh": woh_h, "wor": wor_h, "bo": bo_h, "pert": pert_h, "ident": ident_h,
        })
    return in_maps


def kernel(inputs, w_ih, w_hh, b_ih, b_hh, W_iface, b_iface, W_out, b_out):
    import time as _time
    from concourse import bass_utils

    _t0 = _time.perf_counter()
    key = S
    if key not in _NC_CACHE:
        _NC_CACHE[key] = _build_nc(S)
    nc = _NC_CACHE[key]
    _t1 = _time.perf_counter()
    in_maps = _host_prep(inputs, w_ih, w_hh, b_ih, b_hh, W_iface, b_iface,
                         W_out, b_out)
    _t2 = _time.perf_counter()
    res = bass_utils.run_bass_kernel_spmd(nc, in_maps, core_ids=list(range(NCORES)))
    if os.environ.get("K_PROF"):
        print(f"[kprof] build={_t1-_t0:.1f}s prep={_t2-_t1:.1f}s "
              f"run={_time.perf_counter()-_t2:.1f}s")
    out = np.empty((S, B, OUT), np.float32)
    for c in range(NCORES):
        out[:, c * BL:(c + 1) * BL, :] = res.results[c]["outs"]
    return out

